# revision 32
# baseline (speedup 1.0000x reference)
"""Trainium2 Bass kernel for nn_Attention_7602092114471.

Full multi-head attention block:
  qkv = x @ w_qkv.T ; split q,k,v into 12 heads of d=64
  q = rope(q * d**-0.5) ; k = rope(k)   (lucidrains interleaved RoPE)
  attn = softmax(q @ k.T) ; out = (attn @ v) reassembled, @ w_proj.T + b_proj

Shapes: x [2, 2048, 768], w_qkv [2304, 768], w_proj [768, 768], b_proj [768].

Sharding: 24 (batch, head) pairs -> 8 cores x 3 heads. Core c handles batch
c//4, heads {3g, 3g+1, 3g+2} with g = c%4. Each core computes its heads'
q/k/v projections, attention, and a partial output projection over its
3 heads' feature columns. The host sums the 4 partial projections per batch
(the tensor-parallel all-reduce, done on host during unshard) and adds bias.

Layout (hardcoded for these shapes):
  * x passed transposed per batch (xT [768, 2048], contraction on
    partitions), DMA'd in per-strip chunks so the first projection chain
    starts a few us in instead of after the full 6.3MB load.
  * q/k produced FEATURE-major ([d, token]); scores computed transposed,
    ST[j, i] = k_j . q_i, so the PV matmul (contraction over j) needs no
    transposes anywhere.
  * Softmax: exp on ScalarE straight out of PSUM (constant -8 bias, cancels
    in normalization); the denominator L_i comes for free from the PV matmul
    via 64 ones-columns in the stationary operand (PV output rows 64..127).
    Normalization: both heads' numerator/denominator are copied out of PSUM
    packed into [128, 512] tiles (fast bank release), one DVE reciprocal
    covers two heads, then per-head multiplies. (The ~5x-faster custom-DVE
    reciprocal_approx_fast miscomputes on this runtime - stock ops only.)
  * RoPE: interleaved rotation conjugated into rotate-half-by-32 by
    permuting the q/k weight rows on the host; the swapped-partner
    projection comes from extra (permuted) weight columns, so the rotation
    is 3 DVE ops per [128, 512] tile.
  * v reaches token-major via PE transposes staged through the projection
    PSUM bank (idle during strip 0), scattered by DVE.
  * Dtypes: x / qkv weights / scores fp32r; post-rope q/k, v, exp(scores),
    P, proj weights and output partials bf16 (error budget 2e-2, measured
    4.2e-3).

Single software pipeline (the core of the 314us -> ~216us speedup):
ScalarE exp is the irreducible pacer (12.6M exps/core at 1 elem/lane/cycle
@ 1.2GHz = ~25us per 512-query i-strip). After a ~10us mini-prologue
(strip 0's own k/q2k2/q01 chains), the score/exp stream starts and ALL
remaining work is issued as interleaved PE filler between exp groups so
the PE stays dense (HAM stays at K=8/8) and ScalarE almost never waits:
  * strip 0's fillers: the k/q2k2/q01 projection chains for strips 1-3
    (+ their RoPE on DVE), the v projections and transposes;
  * strip s>=1 fillers: strip s-1's PV chains (heads sequential through a
    2-bank PSUM rotation), its normalization, and the projection of
    strip s-2 (s-1 for the last strip) in a dedicated bank.
Scores for key-strip t only need k of strip t, which is why the exp stream
can start at ~10us. Per key block: one N=1024 exp ACT for h0+h1 (4 PSUM
banks, double-buffered so neither engine waits) and one N=512 ACT for h2
(1 bank, trailing one block). PSUM: 4 + 1 + 2 (chains/PV) + 1 (proj) = 8.
"""

import numpy as np

import concourse.bass as bass
import concourse.mybir as mybir
import concourse.tile as tile
from concourse import bacc, bass_utils
from concourse.masks import make_identity

# Problem constants (hardcoded per contract; kernel.py must be self-contained).
B = 2
N = 2048
C = 768
H = 12
D = 64
ROPE_THETA = 10000.0
NCORES = 8
HPC = 3  # heads per core

F32 = mybir.dt.float32
BF16 = mybir.dt.bfloat16

import os
_BF = lambda name, dflt="1": (BF16 if os.environ.get(name, dflt) == "1" else mybir.dt.float32r)
QK_DT = _BF("K_BF_QK")    # q01/k01/q2d/k2d (score matmuls)
V_DT = _BF("K_BF_V")      # v_sb + e (PV matmul)
P_DT = _BF("K_BF_P")      # P0/P1 + wp (proj matmul)
O_DT = _BF("K_BF_O")      # outT partials
DEBUG_DUMP = os.environ.get("K_DEBUG_DUMP", "0") == "1"

MM_DT = "float32r"

IS = 512                  # strip width for phase-1 projections
NSTRIP = N // IS          # 4
ISA = 512                 # attention i-strip width
NSA = N // ISA
EXPG = 2                  # jb per score group
NJB = N // 128            # 16 key blocks
NGRP = NJB // EXPG        # 8 score groups per strip
KT = C // 128             # 6 contraction tiles for the projections
EXP_BIAS = -8.0           # constant shift inside exp; cancels in normalization


def _mmdt():
    return mybir.dt.float32r if MM_DT == "float32r" else F32


def build_nc():
    """Build the per-core Bass module (same NEFF runs SPMD on all 8 cores)."""
    nc = bacc.Bacc(
        "TRN2",
        target_bir_lowering=False,
        debug=False,
        enable_asserts=False,
    )

    mmdt = _mmdt()
    xT = nc.dram_tensor("xT", [C, N], mmdt, kind="ExternalInput").ap()
    w_feat = nc.dram_tensor("w_feat", [C, 15 * D], mmdt, kind="ExternalInput").ap()
    wp = nc.dram_tensor("wp", [256, C], P_DT, kind="ExternalInput").ap()
    cosT = nc.dram_tensor("cosT", [128, N], F32, kind="ExternalInput").ap()
    sinT = nc.dram_tensor("sinT", [128, N], F32, kind="ExternalInput").ap()
    ones = nc.dram_tensor("ones", [128, D], V_DT, kind="ExternalInput").ap()
    outT = nc.dram_tensor("outT", [C, N], O_DT, kind="ExternalOutput").ap()
    dbg = None
    if DEBUG_DUMP:
        dbg = {
            nm: nc.dram_tensor(f"dbg_{nm}", shp, dt, kind="ExternalOutput").ap()
            for nm, shp, dt in [
                ("q01", [128, N], QK_DT), ("k01", [128, N], QK_DT),
                ("q2d", [128, N], QK_DT), ("k2d", [128, N], QK_DT),
                ("v_sb", [128, NJB * 384], V_DT),
                ("e0", [128, NJB * ISA], V_DT), ("e1", [128, NJB * ISA], V_DT),
                ("e2", [128, NJB * ISA], V_DT),
                ("P0", [128, N], P_DT), ("P1", [128, N], P_DT),
            ]
        }

    with tile.TileContext(nc) as tc:
        _kernel_body(tc, nc, xT, w_feat, wp, cosT, sinT, ones, outT, dbg)
    nc.compile()
    return nc


def _rope_tile(nc, pool, dst, psrc, psrc_s, cos_sb, sin_sb, s):
    """RoPE on one PSUM tile strip: dst = psrc*cos + psrc_s*sinmod (bf16 out,
    both products formed in fp32, one rounding)."""
    rows = psrc.shape[0]
    ss = slice(s * IS, (s + 1) * IS)
    tmp1 = pool.tile([128, IS], F32, name="rope_tmp1", tag="rope_tmp1")
    tmp2 = pool.tile([128, IS], F32, name="rope_tmp2", tag="rope_tmp2")
    nc.vector.tensor_mul(out=tmp1[:rows, :], in0=psrc, in1=cos_sb[:rows, ss])
    nc.vector.tensor_mul(out=tmp2[:rows, :], in0=psrc_s, in1=sin_sb[:rows, ss])
    nc.vector.tensor_add(out=dst[:rows, ss], in0=tmp1[:rows, :], in1=tmp2[:rows, :])


def _kernel_body(tc, nc, xT, w_feat, wp, cosT, sinT, ones, outT, dbg=None):
    import contextlib

    ctx = contextlib.ExitStack()
    with ctx:
        persist = ctx.enter_context(tc.tile_pool(name="persist", bufs=1))
        rope_pool = ctx.enter_context(tc.tile_pool(name="rope", bufs=4))
        attnA = ctx.enter_context(tc.tile_pool(name="attnA", bufs=1))
        # attention-phase PSUM (whole kernel): 4 (h0h1 scores, double-
        # buffered) + 1 (h2) + 2 (chains/PV rotation) + 1 (proj/transposes)
        sts01p = ctx.enter_context(tc.tile_pool(name="sts01", bufs=2, space="PSUM"))
        sts2p = ctx.enter_context(tc.tile_pool(name="sts2", bufs=1, space="PSUM"))
        wkps = ctx.enter_context(tc.tile_pool(name="wkps", bufs=2, space="PSUM"))
        prps = ctx.enter_context(tc.tile_pool(name="prps", bufs=1, space="PSUM"))

        # ---- persistent SBUF tensors -------------------------------------
        q01 = persist.tile([128, N], QK_DT, name="q01")
        k01 = persist.tile([128, N], QK_DT, name="k01")
        q2d = persist.tile([128, N], QK_DT, name="q2d")
        k2d = persist.tile([128, N], QK_DT, name="k2d")
        v_sb = persist.tile([128, NJB, 3 * 128], V_DT, name="v_sb")
        P0 = persist.tile([128, N], P_DT, name="P0")  # heads h0 | h1
        P1 = persist.tile([128, N], P_DT, name="P1")  # h2 duplicated
        wp_sb = persist.tile([128, 2, C], P_DT, name="wp_sb")
        bias_sb = persist.tile([128, 1], F32, name="bias_sb")
        nc.vector.memset(bias_sb, EXP_BIAS)

        # strip-0 exp buffers (live for the whole kernel, allocated before
        # the big phase-1 pool so SBUF peaks stay under the limit)
        e01_0 = attnA.tile([128, NJB, 2, ISA], V_DT, name="e01_0")
        e2_0 = attnA.tile([128, NJB, ISA], V_DT, name="e2_0")

        vtpool = ctx.enter_context(tc.tile_pool(name="vtpool", bufs=1))
        ph1_stack = contextlib.ExitStack()
        ph1 = ph1_stack.enter_context(tc.tile_pool(name="ph1", bufs=1))

        w_sb = ph1.tile([128, KT, 15 * D], _mmdt(), name="w_sb")
        wr = w_feat.rearrange("(o p) f -> p o f", p=128)
        for kt in range(KT):
            nc.sync.dma_start(w_sb[:, kt], wr[:, kt])
        cos_sb = ph1.tile([128, N], F32, name="cos_sb")
        sin_sb = ph1.tile([128, N], F32, name="sin_sb")
        x_sb = [
            ph1.tile([128, N], _mmdt(), name=f"x_sb{kt}", tag=f"x_sb{kt}")
            for kt in range(KT)
        ]
        for s in range(NSTRIP):
            ss = slice(s * IS, (s + 1) * IS)
            for kt in range(KT):
                nc.sync.dma_start(x_sb[kt][:, ss], xT[kt * 128 : (kt + 1) * 128, ss])
            if s == 0:
                nc.sync.dma_start(cos_sb, cosT)
                nc.sync.dma_start(sin_sb, sinT)
        ones_dst = v_sb.rearrange("p j (h x) -> p (j h) x", x=128)[:, :, D:128]
        nc.sync.dma_start(ones_dst, ones[:, None, :].to_broadcast((128, NJB * 3, D)))
        nc.sync.dma_start(wp_sb, wp.rearrange("(o p) f -> p o f", p=128))

        ident = vtpool.tile([128, 128], F32, name="ident")
        make_identity(nc, ident)
        vT01 = vtpool.tile([128, N], F32, name="vT01")
        vT2 = vtpool.tile([64, N], F32, name="vT2")

        # w_feat column blocks (128 wide unless noted):
        #   0: q0|q1   1: swap(q0|q1)   2: k0|k1   3: swap(k0|k1)
        #   4: q2|k2   5: swap(q2|k2)   6: v0|v1   7: v2 (64 wide)
        def qkv_chain(col, m, s):
            ss = slice(s * IS, (s + 1) * IS)
            pt = wkps.tile([128, IS], F32, name="wk", tag="wk")
            for kt in range(KT):
                nc.tensor.matmul(
                    pt[:m, :],
                    w_sb[:, kt, col : col + m],
                    x_sb[kt][:, ss],
                    start=(kt == 0),
                    stop=(kt == KT - 1),
                )
            return pt

        ch_state = {}

        def k_main(s):
            ch_state["k"] = qkv_chain(2 * 128, 128, s)

        def k_swap(s):
            pt_s = qkv_chain(3 * 128, 128, s)
            _rope_tile(nc, rope_pool, k01, ch_state.pop("k"), pt_s,
                       cos_sb, sin_sb, s)

        def qk2_main(s):
            ch_state["qk2"] = qkv_chain(4 * 128, 128, s)

        def qk2_swap(s):
            pt = ch_state.pop("qk2")
            pt_s = qkv_chain(5 * 128, 128, s)
            _rope_tile(nc, rope_pool, q2d, pt[0:64, :], pt_s[0:64, :],
                       cos_sb, sin_sb, s)
            _rope_tile(nc, rope_pool, k2d, pt[64:128, :], pt_s[64:128, :],
                       cos_sb, sin_sb, s)
            # per-strip duplicate rows on ScalarE (idle while filling)
            ss = slice(s * IS, (s + 1) * IS)
            nc.scalar.copy(out=k2d[64:128, ss], in_=k2d[0:64, ss])
            nc.scalar.copy(out=q2d[64:128, ss], in_=q2d[0:64, ss])

        def q_main(s):
            ch_state["q"] = qkv_chain(0, 128, s)

        def q_swap(s):
            pt_s = qkv_chain(128, 128, s)
            _rope_tile(nc, rope_pool, q01, ch_state.pop("q"), pt_s,
                       cos_sb, sin_sb, s)

        # mini-upfront: strip 0's own projections only (~10us)
        k_main(0); k_swap(0)
        qk2_main(0); qk2_swap(0)
        q_main(0); q_swap(0)

        # ---- attention pipeline ------------------------------------------
        e01 = [e01_0, None]
        e2 = [e2_0, None]

        def score_h2(s, jb):
            e2t = e2[s % 2]
            ss = slice(s * ISA, (s + 1) * ISA)
            jbs = slice(jb * 128, (jb + 1) * 128)
            half = jb & 1
            hh = slice(half * 64, half * 64 + 64)
            st2 = sts2p.tile([128, ISA], F32, name="st2", tag="st2")
            nc.tensor.matmul(st2, k2d[hh, jbs], q2d[hh, ss], start=True, stop=True)
            nc.scalar.activation(
                out=e2t[:, jb, :], in_=st2,
                func=mybir.ActivationFunctionType.Exp, bias=bias_sb[:, :],
            )

        def score01(s, jb):
            """h0/h1 scores for one key block + one N=1024 exp ACT.
            sts01 is double-buffered so these MMs never wait on the previous
            ACT and ScalarE never waits on these MMs."""
            e01t = e01[s % 2]
            ss = slice(s * ISA, (s + 1) * ISA)
            jbs = slice(jb * 128, (jb + 1) * 128)
            st01 = sts01p.tile([128, 2, ISA], F32, name="st01", tag="st01")
            nc.tensor.matmul(st01[:, 0, :], k01[0:64, jbs], q01[0:64, ss],
                             start=True, stop=True)
            nc.tensor.matmul(st01[:, 1, :], k01[64:128, jbs], q01[64:128, ss],
                             start=True, stop=True)
            nc.scalar.activation(
                out=e01t[:, jb], in_=st01,
                func=mybir.ActivationFunctionType.Exp, bias=bias_sb[:, :],
            )

        def pv_mms(s, h, pv, jbs):
            e01t, e2t = e01[s % 2], e2[s % 2]
            for jb in jbs:
                mv = e01t[:, jb, h, :] if h < 2 else e2t[:, jb, :]
                nc.tensor.matmul(
                    pv, v_sb[:, jb, h * 128 : (h + 1) * 128], mv,
                    start=(jb == 0), stop=(jb == NJB - 1),
                )

        # ---- strip 0: scores/exp with the rest of phase 1 as filler ------
        def v_chain01(sx):
            ss = slice(sx * IS, (sx + 1) * IS)
            pt = qkv_chain(6 * 128, 128, sx)
            nc.vector.tensor_copy(out=vT01[:, ss], in_=pt[:, :])

        def v_chain2(sx):
            ss = slice(sx * IS, (sx + 1) * IS)
            pt = qkv_chain(7 * 128, 64, sx)
            nc.vector.tensor_copy(out=vT2[:, ss], in_=pt[0:64, :])

        def v_transpose(sx):
            # PE transposes (through the proj bank, idle in strip 0) + DVE
            # scatter into the (v | ones) groups
            for tb in range(4 * sx, 4 * sx + 4):
                tbs = slice(tb * 128, (tb + 1) * 128)
                pp = prps.tile([128, IS], F32, name="pp", tag="pp")
                nc.tensor.transpose(pp[:, 0:128], vT01[:, tbs], ident)
                nc.tensor.transpose(pp[:, 128:192], vT2[:, tbs], ident[0:64, 0:64])
                dst01 = v_sb[:, tb, :].rearrange("p (h x) -> p h x", h=3)[:, 0:2, 0:64]
                nc.vector.tensor_copy(
                    out=dst01,
                    in_=pp[:, 0:128].rearrange("p (h x) -> p h x", h=2))
                nc.vector.tensor_copy(out=v_sb[:, tb, 256:320], in_=pp[:, 128:192])

        q_state = {}

        def q_main(qs):
            q_state["pt"] = qkv_chain(0, 128, qs)

        def q_swap(qs):
            pt_s = qkv_chain(128, 128, qs)
            _rope_tile(nc, rope_pool, q01, q_state["pt"], pt_s, cos_sb, sin_sb, qs)

        s0_fillers = [
            lambda: k_main(1), lambda: k_swap(1),
            lambda: qk2_main(1), lambda: qk2_swap(1),
            lambda: k_main(2), lambda: k_swap(2),
            lambda: qk2_main(2), lambda: qk2_swap(2),
            lambda: k_main(3), lambda: k_swap(3),
            lambda: qk2_main(3), lambda: qk2_swap(3),
            lambda: q_main(1), lambda: q_swap(1),
            lambda: v_chain01(0), lambda: v_chain2(0),
            lambda: q_main(2), lambda: q_swap(2),
            lambda: v_chain01(1), lambda: v_chain2(1),
            lambda: q_main(3), lambda: q_swap(3),
            lambda: (v_chain01(2), v_chain2(2)),
            lambda: (v_chain01(3), v_chain2(3)),
        ]
        fi = 0
        for jb in range(NJB):
            score01(0, jb)
            if jb >= 1:
                score_h2(0, jb - 1)
            if jb == NJB - 1:
                score_h2(0, jb)
            # one filler per group while the k chains race the scores;
            # two per group in the back half so nothing spills past the
            # strip into ScalarE-idle serial time
            for _ in range(1 if jb < 8 else 2):
                if fi < len(s0_fillers):
                    s0_fillers[fi]()
                    fi += 1
        while fi < len(s0_fillers):
            s0_fillers[fi]()
            fi += 1
        ph1_stack.close()

        # ---- strips 1..3 + projections + tail ----------------------------
        with (
            tc.tile_pool(name="attnB", bufs=1) as attnB,
            tc.tile_pool(name="nrm", bufs=2) as nrm,
            tc.tile_pool(name="prout", bufs=4) as prout,
        ):
            e01[1] = attnB.tile([128, NJB, 2, ISA], V_DT, name="e01_1")
            e2[1] = attnB.tile([128, NJB, ISA], V_DT, name="e2_1")

            def norm01_copies(s, pv0, pv1):
                """Copy both heads' PV out of PSUM, packed for one recip:
                rows 0:64 = h0, rows 64:128 = h1 (frees both banks fast)."""
                c01n = nrm.tile([128, ISA], F32, name="c01n", tag="c01n")
                c01d = nrm.tile([128, ISA], F32, name="c01d", tag="c01d")
                nc.vector.tensor_copy(out=c01n[0:64, :], in_=pv0[0:64, :])
                nc.vector.tensor_copy(out=c01d[0:64, :], in_=pv0[64:128, :])
                nc.vector.tensor_copy(out=c01n[64:128, :], in_=pv1[0:64, :])
                nc.vector.tensor_copy(out=c01d[64:128, :], in_=pv1[64:128, :])
                return c01n, c01d

            def norm01_div(s, c01n, c01d, use_act=False):
                ss = slice(s * ISA, (s + 1) * ISA)
                r01 = nrm.tile([128, ISA], F32, name="r01", tag="r01")
                if use_act:
                    # tail only: ScalarE is idle after the last exp, and Ln
                    # shares a table set with Exp -> 1/L = exp(-ln(L))
                    tl = nrm.tile([128, ISA], F32, name="tl", tag="tl")
                    nc.scalar.activation(out=tl, in_=c01d,
                                         func=mybir.ActivationFunctionType.Ln)
                    nc.scalar.activation(out=r01, in_=tl, scale=-1.0,
                                         func=mybir.ActivationFunctionType.Exp)
                else:
                    nc.vector.reciprocal(r01, c01d)
                nc.vector.tensor_mul(out=P0[0:64, ss], in0=c01n[0:64, :],
                                     in1=r01[0:64, :])
                nc.vector.tensor_mul(out=P0[64:128, ss], in0=c01n[64:128, :],
                                     in1=r01[64:128, :])

            def norm2(s, pv, use_act=False):
                ss = slice(s * ISA, (s + 1) * ISA)
                c2n = nrm.tile([64, ISA], F32, name="c2n", tag="c2n")
                c2d = nrm.tile([64, ISA], F32, name="c2d", tag="c2d")
                nc.vector.tensor_copy(out=c2n, in_=pv[0:64, :])
                nc.vector.tensor_copy(out=c2d, in_=pv[64:128, :])
                r2 = nrm.tile([64, ISA], F32, name="r2", tag="r2")
                if use_act:
                    t2 = nrm.tile([64, ISA], F32, name="t2", tag="t2")
                    nc.scalar.activation(out=t2, in_=c2d,
                                         func=mybir.ActivationFunctionType.Ln)
                    nc.scalar.activation(out=r2, in_=t2, scale=-1.0,
                                         func=mybir.ActivationFunctionType.Exp)
                else:
                    nc.vector.reciprocal(r2, c2d)
                nc.vector.tensor_mul(out=P1[0:64, ss], in0=c2n, in1=r2)
                nc.vector.tensor_copy(out=P1[64:128, ss], in_=P1[0:64, ss])

            def proj_obs(s, obs):
                ss = slice(s * IS, (s + 1) * IS)
                for ob in obs:
                    obsl = slice(ob * 128, (ob + 1) * 128)
                    pp = prps.tile([128, IS], F32, name="pp", tag="pp")
                    nc.tensor.matmul(pp, wp_sb[:, 0, obsl], P0[:, ss],
                                     start=True, stop=False)
                    nc.tensor.matmul(pp, wp_sb[:, 1, obsl], P1[:, ss],
                                     start=False, stop=True)
                    ot = prout.tile([128, IS], O_DT, name="ot", tag="ot")
                    nc.vector.tensor_copy(out=ot, in_=pp)
                    nc.sync.dma_start(outT[obsl, ss], ot)

            pvst = {}

            def pv_start(ps, h):
                pv = wkps.tile([128, ISA], F32, name="wk", tag="wk")
                pvst[(ps, h)] = pv
                pv_mms(ps, h, pv, range(0, 8))

            def pv_end(ps, h):
                pv_mms(ps, h, pvst[(ps, h)], range(8, NJB))

            # strips 1..3: PV of strip s-1 (heads sequential, 2-bank
            # rotation) + norms + projections of earlier strips as filler
            nstate = {}
            for s in range(1, NSA):
                ps = s - 1

                def n_copies(ps=ps):
                    nstate["c"] = norm01_copies(ps, pvst.pop((ps, 0)),
                                                pvst.pop((ps, 1)))

                def n_div(ps=ps):
                    norm01_div(ps, *nstate.pop("c"))

                fillers = []
                if s == 1:
                    fillers += [lambda sx=sx: v_transpose(sx) for sx in range(4)]
                fillers += [
                    lambda ps=ps: pv_start(ps, 0), lambda ps=ps: pv_end(ps, 0),
                    lambda ps=ps: pv_start(ps, 1), lambda ps=ps: pv_end(ps, 1),
                    n_copies,
                    lambda ps=ps: pv_start(ps, 2), lambda ps=ps: pv_end(ps, 2),
                    n_div,
                    lambda ps=ps: norm2(ps, pvst.pop((ps, 2))),
                ]
                if s >= 2:
                    fillers += [lambda ob=ob, t=s - 2: proj_obs(t, [2 * ob, 2 * ob + 1])
                                for ob in range(3)]
                if s == NSA - 1:
                    fillers += [lambda ob=ob, t=s - 1: proj_obs(t, [2 * ob, 2 * ob + 1])
                                for ob in range(3)]
                    # start the last strip's h0 PV early (its exps are done
                    # through jb14 by the final group; bank freed by the
                    # norm copies above)
                    def pv30():
                        pv = wkps.tile([128, ISA], F32, name="wk", tag="wk")
                        pvst[(s, 0)] = pv
                        pv_mms(s, 0, pv, range(0, 15))
                    fillers.append(pv30)
                fi = 0
                for jb in range(NJB):
                    score01(s, jb)
                    if jb >= 1:
                        score_h2(s, jb - 1)
                    if jb == NJB - 1:
                        score_h2(s, jb)
                    if fi < len(fillers):
                        fillers[fi]()
                        fi += 1
                while fi < len(fillers):
                    fillers[fi]()
                    fi += 1

            # tail: finish the last strip
            s = NSA - 1
            pv_mms(s, 0, pvst[(s, 0)], [15])
            pv_start(s, 1)
            pv_end(s, 1)
            c = norm01_copies(s, pvst.pop((s, 0)), pvst.pop((s, 1)))
            pv_start(s, 2)
            pv_end(s, 2)
            norm01_div(s, *c)
            norm2(s, pvst.pop((s, 2)))
            proj_obs(s, range(0, 6))


# ---------------------------------------------------------------------------
# Host-side sharding / unsharding
# ---------------------------------------------------------------------------

def _rope_tables():
    inv_freq = 1.0 / (ROPE_THETA ** (np.arange(0, D, 2, dtype=np.float64) / D))
    ang = np.arange(N, dtype=np.float64)[None, :] * inv_freq[:, None]  # [32, N]
    cos64 = np.concatenate([np.cos(ang), np.cos(ang)], axis=0)
    sin64 = np.concatenate([-np.sin(ang), np.sin(ang)], axis=0)
    cosT = np.concatenate([cos64, cos64], axis=0).astype(np.float32)
    sinT = np.concatenate([sin64, sin64], axis=0).astype(np.float32)
    return cosT, sinT


def _conv(a, dt):
    """Convert fp32 array for a device tensor of dtype dt."""
    import ml_dtypes

    a = np.ascontiguousarray(a, dtype=np.float32)
    return a.astype(ml_dtypes.bfloat16) if dt == BF16 else a


def make_core_inputs(x, w_qkv, w_proj):
    """Build the 8 per-core input dicts from full inputs."""
    x = np.asarray(x, dtype=np.float32)
    w_qkv = np.asarray(w_qkv, dtype=np.float32)
    w_proj = np.asarray(w_proj, dtype=np.float32)

    cosT, sinT = _rope_tables()
    perm = np.concatenate([np.arange(0, D, 2), np.arange(1, D, 2)])  # de-interleave
    wq, wk, wv = w_qkv[0:C], w_qkv[C : 2 * C], w_qkv[2 * C : 3 * C]
    scale = np.float32(D ** -0.5)
    wpT = np.ascontiguousarray(w_proj.T)  # [in_features, out_channels]

    in_maps = []
    for c in range(NCORES):
        b, g = divmod(c, 4)
        h0, h1, h2 = 3 * g, 3 * g + 1, 3 * g + 2

        def qrow(h):
            return wq[h * D : (h + 1) * D][perm] * scale

        def krow(h):
            return wk[h * D : (h + 1) * D][perm]

        def vrow(h):
            return wv[h * D : (h + 1) * D]

        def swap32(w64):
            # rows permuted by the rotate-half partner p ^ 32
            return np.concatenate([w64[32:64], w64[0:32]], axis=0)

        blocks = [qrow(h0), qrow(h1)]
        blocks += [swap32(qrow(h0)), swap32(qrow(h1))]
        blocks += [krow(h0), krow(h1)]
        blocks += [swap32(krow(h0)), swap32(krow(h1))]
        blocks += [qrow(h2), krow(h2)]
        blocks += [swap32(qrow(h2)), swap32(krow(h2))]
        blocks += [vrow(h0), vrow(h1), vrow(h2)]
        w_feat = np.concatenate(blocks, axis=0).T  # [C, 15*D]
        wp_rows = np.concatenate(
            [wpT[h0 * D : (h0 + 1) * D], wpT[h1 * D : (h1 + 1) * D],
             0.5 * wpT[h2 * D : (h2 + 1) * D], 0.5 * wpT[h2 * D : (h2 + 1) * D]],
            axis=0,
        )  # [256, C]
        in_maps.append(
            {
                "xT": np.ascontiguousarray(x[b].T),
                "w_feat": np.ascontiguousarray(w_feat, dtype=np.float32),
                "wp": _conv(wp_rows, P_DT),
                "cosT": cosT,
                "sinT": sinT,
                "ones": _conv(np.ones((128, D), dtype=np.float32), V_DT),
            }
        )
    return in_maps


def unshard(core_outs, b_proj):
    """Sum the 4 partial projections per batch, transpose, add bias."""
    b_proj = np.asarray(b_proj, dtype=np.float32)
    out = np.empty((B, N, C), dtype=np.float32)
    for b in range(B):
        acc = np.asarray(core_outs[4 * b], dtype=np.float32).copy()
        for g in range(1, 4):
            acc += np.asarray(core_outs[4 * b + g], dtype=np.float32)
        out[b] = acc.T + b_proj
    return out


_NC_CACHE = {}


def get_nc():
    key = (MM_DT, QK_DT, V_DT, P_DT, O_DT, DEBUG_DUMP)
    if key not in _NC_CACHE:
        _NC_CACHE[key] = build_nc()
    return _NC_CACHE[key]


def run(inputs, trace=False, **spmd_kwargs):
    """Run on hardware; returns (full_output, BassKernelResults)."""
    nc = get_nc()
    in_maps = make_core_inputs(inputs["x"], inputs["w_qkv"], inputs["w_proj"])
    res = bass_utils.run_bass_kernel_spmd(
        nc, in_maps, core_ids=list(range(NCORES)), trace=trace, **spmd_kwargs
    )
    core_outs = [r["outT"] for r in res.results]
    return unshard(core_outs, inputs["b_proj"]), res


def kernel(x, w_qkv, w_proj, b_proj):
    out, _ = run({"x": x, "w_qkv": w_qkv, "w_proj": w_proj, "b_proj": b_proj})
    return out
